# revision 1
# baseline (speedup 1.0000x reference)
"""BlockCirculantConv on 8 Trainium2 NeuronCores.

The reference computes, per batch image b:
    xu = unfold(x[b])                       # (2304, 1024), f = c*9 + (di*3+dj)
    Y  = xu.flatten().reshape(1024, 2304)   # torch-faithful row-major reshape
    out_T = (Y @ W).T                       # W = expanded block-circulant (2304, 512)
    out[b] = out_T.reshape(512, 32, 32)
with W[q*64+s, p*64+t] = weight[p, q, (t-s) % 64]  (rfft product == circular conv).

Because of the row-major reshape, row n = 4c+j of Y is a contiguous 2304-chunk of
channel c's 9 shifted images:  Y[4c+j, k] = Z_c[(j*2304+k)//1024, (j*2304+k)%1024]
where Z_c[dd, i*32+jj] = xpad[b, c, i+dd//3-1, jj+dd%3-1].

So out_T[m, 4c+j] = sum_k W[k, m] * S_kj[k, c]  where for a 128-aligned k-tile the
rhs S tile is a CONTIGUOUS 128-row slice of a (per-dj) zero-padded, transposed copy
of the image: xt3[dj, 1 + i*32 + jj, c] = xpad[b, c, i-1, jj+dj-1].

Device kernel per core (data-parallel over batch, 1 image/core):
  - inputs in fp16 (halves DMA bytes; fp32 PSUM accumulate; rel err ~3e-4)
  - weights + all rhs data DMA'd into SBUF as a few large chunk transfers
    (each dma_start costs ~650ns of HWDGE sequencer time)
  - 8 PSUM banks accumulate out_T as 4 m-tiles x 2 column-halves over 18
    k-tiles; dummy warm-up matmuls release the HAM clock gate early;
    k-tiles 10..17 run one psum at a time so drains overlap the stream
  - drain: DVE/ACT copies PSUM -> SBUF, DMA out in (j*256+c) column
    order; host permutes columns back to n = 4c+j.
"""

import sys

if "/opt/trn_rl_repo" not in sys.path:
    sys.path.insert(0, "/opt/trn_rl_repo")

import numpy as np

B, C, H, W_IMG = 8, 256, 32, 32
L = H * W_IMG               # 1024
BLK = 64
Q, P = 36, 8
K_FULL = Q * BLK            # 2304
M_OUT = P * BLK             # 512
KT = K_FULL // 128          # 18 k-tiles
N_CORES = 8
XT_ROWS = 1 + 34 * 32 + 1   # 1090 padded rows per dj copy

_CACHE = {}

# "float16" (half input bytes, full-rate PE, rel err ~3e-4) or
# "float32r" (single-pass fp32 matmul, rel err ~1.5e-4)
IN_DTYPE = "float16"


def _patch_ldw_opt():
    """(kept as a hook; ldw-opt=true fails walrus codegen, so this is a no-op)"""
    from concourse import bass_utils

    if getattr(bass_utils.run_command, "_ldw_patched", False):
        return
    orig = bass_utils.run_command

    def run_command(cmd, *a, **kw):
        cmd = [
            c
            if isinstance(c, str)
            else c
            for c in cmd
        ]
        return orig(cmd, *a, **kw)

    run_command._ldw_patched = True
    bass_utils.run_command = run_command


def _build_nc():
    import concourse.bacc as bacc
    import concourse.tile as tile
    import concourse.mybir as mybir

    _patch_ldw_opt()

    dt = mybir.dt
    din = getattr(dt, IN_DTYPE)
    nc = bacc.Bacc("TRN2", target_bir_lowering=False, debug=False)

    xt3 = nc.dram_tensor("xt3", [3, XT_ROWS, C], din, kind="ExternalInput").ap()
    wmat = nc.dram_tensor("wmat", [K_FULL, M_OUT], din, kind="ExternalInput").ap()
    out = nc.dram_tensor("out", [M_OUT, L], dt.float32, kind="ExternalOutput").ap()

    f32 = dt.float32

    # S chunk plan: for each j, the u-range [j*2304, (j+1)*2304) splits at
    # dd (=u//1024) boundaries into runs of whole k-tiles with a constant
    # source row offset. Each dma_start costs ~650ns of HWDGE sequencer
    # time, so use as few (big) chunks as possible; only the first k-tiles
    # get a small chunk so the PE can start early.
    chunks = []  # (j, kt_start, n_kt, dj, src_row0)
    for j in range(4):
        kt = 0
        while kt < KT:
            u = j * K_FULL + kt * 128
            dd, l0 = divmod(u, L)
            di, dj = divmod(dd, 3)
            kt_end_dd = min(KT, ((dd + 1) * L - j * K_FULL) // 128)
            cap = 2 if kt == 0 else (4 if kt <= 6 else KT)
            n_kt = min(cap, kt_end_dd - kt)
            chunks.append((j, kt, n_kt, dj, 1 + di * 32 + l0))
            kt += n_kt
    # issue order: ascending kt so early k-tiles land first
    chunks.sort(key=lambda c: (c[1], c[0]))
    # W chunk plan: (kt_start, n_kt)
    wchunks = [(0, 2), (2, 4), (6, 4), (10, 4), (14, 4)]

    with tile.TileContext(nc) as tc:
        with (
            tc.tile_pool(name="wpool", bufs=1) as wpool,
            tc.tile_pool(name="spool", bufs=1) as spool,
            tc.tile_pool(name="opool", bufs=4) as opool,
            tc.tile_pool(name="ppool", bufs=1, space="PSUM") as ppool,
        ):
            # PE warmup: the HAM clock gate starts at 1.2 GHz and needs
            # ~3.4us of sustained PE activity to release to 2.4 GHz. Run
            # dummy matmuls on a zeroed tile while the first DMA chunks are
            # still in flight so the real matmuls start warm.
            wz = wpool.tile([128, 512], din, name="wz", tag="wz")
            nc.gpsimd.memset(wz[:], 0.0)

            # 8 PSUM accumulators: index = mt*2 + nh (m-tile x column-half)
            psums = [
                ppool.tile([128, 512], f32, name=f"ps{i}", tag=f"ps{i}")
                for i in range(8)
            ]

            # All rhs data resident: sbig[p, kt, j, c]; weights wbig[p, kt, m]
            sbig = spool.tile([128, KT, 4, 256], din, name="sbig", tag="sbig")
            wbig = wpool.tile([128, KT, 512], din, name="wbig", tag="wbig")

            for _ in range(8):
                nc.tensor.matmul(
                    psums[7][:], wz[:, :128], wz[:], start=True, stop=True
                )

            # S chunks on the sync ring, W chunks on the scalar ring,
            # both in ascending-kt order
            # The kt0 chunks + w0 gate the first matmul; split their
            # triggers across both HWDGE rings (sync + scalar) so the
            # ~650ns-per-trigger serialization doesn't stack up.
            def issue_s(c, eng):
                j, kt0, n_kt, dj, r0 = c
                src = xt3[dj, r0 : r0 + n_kt * 128, :].rearrange(
                    "(blk p) c -> p blk c", p=128
                )
                eng.dma_start(sbig[:, kt0 : kt0 + n_kt, j, :], src)

            first = [c for c in chunks if c[1] == 0]
            rest = [c for c in chunks if c[1] > 0]
            issue_s(first[0], nc.sync)
            issue_s(first[1], nc.sync)
            issue_s(first[2], nc.scalar)
            issue_s(first[3], nc.scalar)

            ci = 0
            for kt0w, n_ktw in wchunks:
                while ci < len(rest) and rest[ci][1] <= kt0w:
                    issue_s(rest[ci], nc.sync)
                    ci += 1
                wsrc = wmat[kt0w * 128 : (kt0w + n_ktw) * 128, :].rearrange(
                    "(blk p) m -> p blk m", p=128
                )
                nc.scalar.dma_start(wbig[:, kt0w : kt0w + n_ktw, :], wsrc)
            for c in rest[ci:]:
                issue_s(c, nc.sync)

            # Phase 1: k-tiles 0..SPLIT-1 round-robin over all 8 psums
            # (keeps every accumulator fed while chunks stream in).
            # Phase 2: once all data is resident, finish one psum at a
            # time so drains + output stores overlap the remaining
            # matmuls instead of piling up in the tail.
            SPLIT = 10
            for kt in range(SPLIT):
                for mt in range(4):
                    for nh in range(2):
                        nc.tensor.matmul(
                            psums[mt * 2 + nh][:],
                            wbig[:, kt, mt * 128 : (mt + 1) * 128],
                            sbig[:, kt, nh * 2 : nh * 2 + 2, :],
                            start=(kt == 0),
                            stop=False,
                        )
            for mt in range(4):
                for nh in range(2):
                    for kt in range(SPLIT, KT):
                        nc.tensor.matmul(
                            psums[mt * 2 + nh][:],
                            wbig[:, kt, mt * 128 : (mt + 1) * 128],
                            sbig[:, kt, nh * 2 : nh * 2 + 2, :],
                            start=False,
                            stop=(kt == KT - 1),
                        )

            # Drain: contiguous copies; out stays in (j*256+c) column order,
            # host permutes to n = 4c+j. Per-half DMAs so the final store
            # pipelines behind the last copies.
            for mt in range(4):
                ot = opool.tile([128, L], f32, name="ot", tag="ot")
                for nh in range(2):
                    src = psums[mt * 2 + nh][:]
                    dst = ot[:, nh * 512 : (nh + 1) * 512]
                    if nh == 0:
                        nc.vector.tensor_copy(dst, src)
                    else:
                        nc.scalar.copy(dst, src)
                    nc.sync.dma_start(
                        out[mt * 128 : (mt + 1) * 128, nh * 512 : (nh + 1) * 512],
                        dst,
                    )

    nc.compile()
    return nc


def _host_prep(x, weight):
    np_in = np.float16 if IN_DTYPE == "float16" else np.float32
    x = np.ascontiguousarray(x, dtype=np.float32)
    weight = np.ascontiguousarray(weight, dtype=np.float32)

    # Expanded block-circulant matrix: W[q*64+s, p*64+t] = weight[p, q, (t-s)%64]
    idx = (np.arange(BLK)[None, :] - np.arange(BLK)[:, None]) % BLK   # (s, t)
    w4 = weight[:, :, idx]                                            # (p, q, s, t)
    wmat = np.ascontiguousarray(
        w4.transpose(1, 2, 0, 3).reshape(K_FULL, M_OUT), dtype=np_in
    )

    # Per-batch padded/shifted transposed images: xt3[b, dj, 1+i*32+jj, c]
    #   = x[b, c, i-1, jj+dj-1] (zero outside the image)
    xp = x.transpose(0, 2, 3, 1).astype(np_in)                        # (b, i, j, c)
    xt3 = np.zeros((B, 3, XT_ROWS, C), np_in)
    v = xt3[:, :, 1 : 1 + 34 * 32, :].reshape(B, 3, 34, 32, C)
    v[:, 0, 1:33, 1:32] = xp[:, :, 0:31]
    v[:, 1, 1:33, 0:32] = xp
    v[:, 2, 1:33, 0:31] = xp[:, :, 1:32]
    return xt3, wmat


def _run(x, weight, trace=False, trace_kwargs=None):
    from concourse.bass_utils import run_bass_kernel_spmd

    if "nc" not in _CACHE:
        _CACHE["nc"] = _build_nc()
    nc = _CACHE["nc"]

    xt3, wmat = _host_prep(x, weight)
    in_maps = [{"xt3": xt3[b], "wmat": wmat} for b in range(N_CORES)]
    res = run_bass_kernel_spmd(
        nc,
        in_maps,
        list(range(N_CORES)),
        trace=trace,
        **(trace_kwargs or {}),
    )
    out = np.stack([res.results[b]["out"] for b in range(N_CORES)])
    # device columns are (j*256 + c); output spatial index is n = 4c + j
    out = (
        out.reshape(B, M_OUT, 4, 256)
        .transpose(0, 1, 3, 2)
        .reshape(B, M_OUT, H, W_IMG)
    )
    return np.ascontiguousarray(out), res


def kernel(x, weight):
    out, _ = _run(x, weight, trace=False)
    return out



# revision 2
# speedup vs baseline: 1.1855x; 1.1855x over previous
"""BlockCirculantConv on 8 Trainium2 NeuronCores.

The reference computes, per batch image b:
    xu = unfold(x[b])                       # (2304, 1024), f = c*9 + (di*3+dj)
    Y  = xu.flatten().reshape(1024, 2304)   # torch-faithful row-major reshape
    out_T = (Y @ W).T                       # W = expanded block-circulant (2304, 512)
    out[b] = out_T.reshape(512, 32, 32)
with W[q*64+s, p*64+t] = weight[p, q, (t-s) % 64]  (rfft product == circular conv).

Row n = 4c+j of Y is a contiguous 2304-chunk of channel c's 9 shifted images, so
the rhs S matrix S[k, j*256+c] (k = contraction row) is a gather of zero-padded
shifted images.  v2: the gather is done ON HOST into the exact SBUF layout
sin[p, kt, j, c] (p = k%128, kt = k//128), so the device does 5 large contiguous
DMAs instead of ~27 strided chunk transfers (each dma_start costs ~650-950ns of
HWDGE sequencer time).  Weights likewise pre-tiled to win[p, kt, m].

Device kernel per core (data-parallel over batch, 1 image/core):
  - inputs fp16 (halves DMA bytes; fp32 PSUM accumulate; rel err ~3e-4)
  - 8 PSUM banks accumulate out_T as 4 m-tiles x 2 column-halves over 18
    k-tiles; 16 small dummy matmuls on a zeroed tile release the HAM clock
    gate (~3.4us ramp) while the first DMA chunks are in flight
  - phase 1 (kt 0..9) round-robins all 8 psums so any resident kt makes
    progress; phase 2 (kt 10..17) finishes one psum at a time so DVE
    drains + fp16 output stores overlap the remaining matmul stream
  - output stored fp16 in (j*256+c) column order; host permutes columns
    back to n = 4c+j and casts to fp32.
"""

import sys

if "/opt/trn_rl_repo" not in sys.path:
    sys.path.insert(0, "/opt/trn_rl_repo")

import numpy as np

B, C, H, W_IMG = 8, 256, 32, 32
L = H * W_IMG               # 1024
BLK = 64
Q, P = 36, 8
K_FULL = Q * BLK            # 2304
M_OUT = P * BLK             # 512
KT = K_FULL // 128          # 18 k-tiles
N_CORES = 8
XT_ROWS = 1 + 34 * 32 + 1   # 1090 padded rows per dj copy

_CACHE = {}

SPLIT = 10                  # kt phase boundary: round-robin -> psum-major
S_CHUNKS = [(0, 1), (1, 3), (3, 7), (7, 12), (12, 18)]   # sync ring
W_CHUNKS = [(0, 1), (1, 4), (4, 10), (10, 18)]           # scalar ring
N_WARM = 16                 # dummy matmuls, N=256 cold ~216ns each ≈ 3.5us


def _build_nc():
    import concourse.bacc as bacc
    import concourse.tile as tile
    import concourse.mybir as mybir

    dt = mybir.dt
    f16 = dt.float16
    f32 = dt.float32
    nc = bacc.Bacc("TRN2", target_bir_lowering=False, debug=False)

    sin = nc.dram_tensor("sin", [128, KT, 4, 256], f16, kind="ExternalInput").ap()
    win = nc.dram_tensor("win", [128, KT, M_OUT], f16, kind="ExternalInput").ap()
    out = nc.dram_tensor("out", [M_OUT, L], f16, kind="ExternalOutput").ap()

    with tile.TileContext(nc) as tc:
        with (
            tc.tile_pool(name="pool", bufs=1) as pool,
            tc.tile_pool(name="ppool", bufs=1, space="PSUM") as ppool,
        ):
            # Input DMAs first: big contiguous chunks, smallest first so the
            # first k-tile lands while the warmup matmuls ramp the clock.
            sbig = pool.tile([128, KT, 4, 256], f16, name="sbig", tag="sbig")
            wbig = pool.tile([128, KT, M_OUT], f16, name="wbig", tag="wbig")
            for a, b in S_CHUNKS:
                nc.sync.dma_start(sbig[:, a:b, :, :], sin[:, a:b, :, :])
            for a, b in W_CHUNKS:
                nc.scalar.dma_start(wbig[:, a:b, :], win[:, a:b, :])

            # PE warmup: HAM clock gate needs ~3.4us of sustained PE activity
            # to go 1.2 -> 2.4 GHz; run dummies while the DMAs stream.
            wz = pool.tile([128, 256], f16, name="wz", tag="wz")
            nc.gpsimd.memset(wz[:], 0.0)

            psums = [
                ppool.tile([128, 512], f32, name=f"ps{i}", tag=f"ps{i}")
                for i in range(8)
            ]
            for _ in range(N_WARM):
                nc.tensor.matmul(
                    psums[7][:, :256], wz[:, :128], wz[:], start=True, stop=True
                )

            # Phase 1: kt 0..SPLIT-1 round-robin over all 8 psums.
            for kt in range(SPLIT):
                for mt in range(4):
                    for nh in range(2):
                        nc.tensor.matmul(
                            psums[mt * 2 + nh][:],
                            wbig[:, kt, mt * 128 : (mt + 1) * 128],
                            sbig[:, kt, nh * 2 : nh * 2 + 2, :],
                            start=(kt == 0),
                            stop=False,
                        )
            # Phase 2: finish one psum at a time; DVE drain + fp16 store
            # overlap the remaining matmuls.
            for mt in range(4):
                for nh in range(2):
                    for kt in range(SPLIT, KT):
                        nc.tensor.matmul(
                            psums[mt * 2 + nh][:],
                            wbig[:, kt, mt * 128 : (mt + 1) * 128],
                            sbig[:, kt, nh * 2 : nh * 2 + 2, :],
                            start=False,
                            stop=(kt == KT - 1),
                        )
                    ot = pool.tile(
                        [128, 512], f16, name=f"ot{mt}{nh}", tag=f"ot{mt}{nh}"
                    )
                    nc.vector.tensor_copy(ot[:], psums[mt * 2 + nh][:])
                    nc.scalar.dma_start(
                        out[mt * 128 : (mt + 1) * 128, nh * 512 : (nh + 1) * 512],
                        ot[:],
                    )

    nc.compile()
    return nc


def _host_prep(x, weight):
    x = np.ascontiguousarray(x, dtype=np.float32)
    weight = np.ascontiguousarray(weight, dtype=np.float32)

    # Expanded block-circulant matrix: W[q*64+s, p*64+t] = weight[p, q, (t-s)%64]
    idx = (np.arange(BLK)[None, :] - np.arange(BLK)[:, None]) % BLK   # (s, t)
    w4 = weight[:, :, idx]                                            # (p, q, s, t)
    wmat = w4.transpose(1, 2, 0, 3).reshape(K_FULL, M_OUT).astype(np.float16)
    win = np.ascontiguousarray(
        wmat.reshape(KT, 128, M_OUT).transpose(1, 0, 2)
    )                                                                 # (p, kt, m)

    # Shifted zero-padded transposed images: xt3[b, dj, 1+r*32+s, c]
    #   = x[b, c, r-1, s-1+dj] (zero outside the image)
    xp = x.transpose(0, 2, 3, 1).astype(np.float16)                   # (b, i, j, c)
    xt3 = np.zeros((B, 3, XT_ROWS, C), np.float16)
    v = xt3[:, :, 1 : 1 + 34 * 32, :].reshape(B, 3, 34, 32, C)
    v[:, 0, 1:33, 1:32] = xp[:, :, 0:31]
    v[:, 1, 1:33, 0:32] = xp
    v[:, 2, 1:33, 0:31] = xp[:, :, 1:32]

    # Gather into the device SBUF layout sin[b, p, kt, j, c]:
    #   k = kt*128+p, t = j*2304+k, dd = t//1024, l = t%1024,
    #   sin[...] = xt3[b, dd%3, 1 + (dd//3)*32 + l, c]
    t = np.arange(4)[None, :] * K_FULL + np.arange(K_FULL)[:, None]   # (k, j)
    dd, l = divmod(t, L)
    row = 1 + (dd // 3) * 32 + l
    vals = xt3[:, dd % 3, row, :]                                     # (b, k, j, c)
    sin = np.ascontiguousarray(
        vals.reshape(B, KT, 128, 4, C).transpose(0, 2, 1, 3, 4)
    )                                                                 # (b, p, kt, j, c)
    return sin, win


def _run(x, weight, trace=False, trace_kwargs=None):
    from concourse.bass_utils import run_bass_kernel_spmd

    if "nc" not in _CACHE:
        _CACHE["nc"] = _build_nc()
    nc = _CACHE["nc"]

    sin, win = _host_prep(x, weight)
    in_maps = [{"sin": sin[b], "win": win} for b in range(N_CORES)]
    res = run_bass_kernel_spmd(
        nc,
        in_maps,
        list(range(N_CORES)),
        trace=trace,
        **(trace_kwargs or {}),
    )
    out = np.stack([res.results[b]["out"] for b in range(N_CORES)])
    # device columns are (j*256 + c); output spatial index is n = 4c + j
    out = (
        out.reshape(B, M_OUT, 4, 256)
        .transpose(0, 1, 3, 2)
        .reshape(B, M_OUT, H, W_IMG)
        .astype(np.float32)
    )
    return np.ascontiguousarray(out), res


def kernel(x, weight):
    out, _ = _run(x, weight, trace=False)
    return out
